# revision 3
# baseline (speedup 1.0000x reference)
"""Performer (FAVOR+) multi-head fast-attention TRN2 kernel — self-contained.

Problem: B=4, N=4096, D=1024, H=16, M=256, DH=64.
Sharding: 2 heads per core (head-parallel attention) on 8 NeuronCores;
on-device AllToAll re-shards to sequence-parallel for the output Linear
(row-parallel, no partial sums); host stitches the 8 n-shards.

All Performer stabilizers that cancel in the num/den ratio are dropped
on device; the k-side row max and ||k||^2 factors are folded into v, so
the result matches the reference exactly up to float rounding.
"""
import contextlib
import sys

sys.path.insert(0, "/opt/trn_rl_repo")

import numpy as np

import concourse.bacc as bacc
import concourse.mybir as mybir
from concourse.tile import TileContext
from concourse.bass_utils import run_bass_kernel_spmd

F32 = mybir.dt.float32
F32R = mybir.dt.float32r
AF = mybir.ActivationFunctionType
ALU = mybir.AluOpType

NCORES = 8
B, N, D = 4, 4096, 1024
H, M, DH = 16, 256, 64
T = N // 128
J = N // 512
NS = N // NCORES
DS = float(DH) ** -0.25

_CACHE = {}


def _build():
    nc = bacc.Bacc(num_devices=NCORES)
    groups = [list(range(NCORES))]

    qT = nc.declare_dram_parameter("qT", [B, 2, DH, N], F32, isOutput=False)
    kT = nc.declare_dram_parameter("kT", [B, 2, DH, N], F32, isOutput=False)
    kn = nc.declare_dram_parameter("kn", [B, T, 128, 128], F32, isOutput=False)
    vn = nc.declare_dram_parameter("vn", [B, T, 128, 128], F32, isOutput=False)
    projT2 = nc.declare_dram_parameter("projT2", [128, M], F32, isOutput=False)
    WT = nc.declare_dram_parameter("WT", [D, D], F32, isOutput=False)
    ident = nc.declare_dram_parameter("ident", [128, 128], F32, isOutput=False)
    out_ext = nc.declare_dram_parameter("out", [B, NS, D], F32, isOutput=True)

    h_in = nc.dram_tensor("h_in", [B, NCORES, 130, NS], F32)
    h_out = nc.dram_tensor("h_out", [B, NCORES, 130, NS], F32)
    dinv_scr = nc.dram_tensor("dinv_scr", [B, 2 * NCORES * NS], F32)
    den_scr = nc.dram_tensor("den_scr", [B, 2 * NCORES * NS], F32)

    with TileContext(nc) as tc:
        with contextlib.ExitStack() as stk:
            const_p = stk.enter_context(tc.tile_pool(name="const", bufs=1))
            qkT_p = stk.enter_context(tc.tile_pool(name="qkT", bufs=2))
            knv_p = stk.enter_context(tc.tile_pool(name="knv", bufs=1))
            ek_p = stk.enter_context(tc.tile_pool(name="ek", bufs=1))
            small_p = stk.enter_context(tc.tile_pool(name="small", bufs=3))
            vaug_p = stk.enter_context(tc.tile_pool(name="vaug", bufs=1))
            qpt_p = stk.enter_context(tc.tile_pool(name="qpt", bufs=2))
            stag_p = stk.enter_context(tc.tile_pool(name="stag", bufs=2))
            lin_p = stk.enter_context(tc.tile_pool(name="lin", bufs=1))
            outc_p = stk.enter_context(tc.tile_pool(name="outc", bufs=3))
            ps_k = stk.enter_context(tc.tile_pool(name="psk", bufs=1, space="PSUM"))
            ps_q = stk.enter_context(tc.tile_pool(name="psq", bufs=2, space="PSUM"))
            ps_ctx = stk.enter_context(tc.tile_pool(name="psctx", bufs=1, space="PSUM"))
            ps_o = stk.enter_context(tc.tile_pool(name="pso", bufs=2, space="PSUM"))
            ps_lin = stk.enter_context(tc.tile_pool(name="pslin", bufs=2, space="PSUM"))

            projT2_sb = const_p.tile([128, M], F32R, tag="projT2")
            nc.sync.dma_start(out=projT2_sb[:], in_=projT2[:].bitcast(F32R))
            ident_sb = const_p.tile([128, 128], F32, tag="ident")
            nc.sync.dma_start(out=ident_sb[:], in_=ident[:])
            WT_sb = const_p.tile([128, NCORES, D], F32R, tag="WT")
            nc.sync.dma_start(out=WT_sb[:],
                              in_=WT[:].rearrange("(cc p) o -> p cc o", p=128).bitcast(F32R))

            for b in range(B):
                kn_sb = knv_p.tile([128, T, 128], F32, tag="kn")
                nc.sync.dma_start(out=kn_sb[:], in_=kn[b].rearrange("t p d -> p t d"))
                v_sb = knv_p.tile([128, T, 128], F32, tag="v")
                nc.sync.dma_start(out=v_sb[:], in_=vn[b].rearrange("t p d -> p t d"))

                kflat = kn_sb[:].rearrange("p t d -> p (t d)")
                nc.vector.scalar_tensor_tensor(
                    out=kflat, in0=kflat, scalar=0.0, in1=kflat,
                    op0=ALU.add, op1=ALU.mult)
                dn_raw = small_p.tile([128, T, 2], F32, tag="dn")
                nc.vector.tensor_reduce(
                    out=dn_raw[:],
                    in_=kn_sb[:].rearrange("p t (h d) -> p t h d", h=2),
                    axis=mybir.AxisListType.X, op=ALU.add)

                for h in range(2):
                    qkT_sb = qkT_p.tile([128, N], F32R, tag="qkT")
                    nc.sync.dma_start(out=qkT_sb[0:DH, :], in_=kT[b, h].bitcast(F32R))
                    nc.sync.dma_start(out=qkT_sb[DH:128, :], in_=qT[b, h].bitcast(F32R))

                    ek_sb = ek_p.tile([128, T, M], F32R, tag="ek")
                    for t in range(T):
                        pk = ps_k.tile([128, M], F32, tag="pk")
                        nc.tensor.matmul(
                            pk[:], qkT_sb[0:DH, 128 * t:128 * (t + 1)],
                            projT2_sb[0:DH, :],
                            start=True, stop=True, skip_group_check=True)
                        nc.scalar.activation(ek_sb[:, t, :], pk[:], AF.Exp, scale=DS)

                    me = small_p.tile([128, T], F32, tag="me")
                    nc.vector.tensor_reduce(out=me[:], in_=ek_sb[:],
                                            axis=mybir.AxisListType.X, op=ALU.max)
                    eg = small_p.tile([128, T], F32, tag="eg")
                    nc.scalar.activation(eg[:], dn_raw[:, :, h], AF.Exp,
                                         scale=-0.5 * DS * DS)
                    rme = small_p.tile([128, T], F32, tag="rme")
                    nc.vector.reciprocal(rme[:], me[:])
                    g = small_p.tile([128, T], F32, tag="g")
                    nc.vector.tensor_tensor(out=g[:], in0=eg[:], in1=rme[:],
                                            op=ALU.mult)

                    vaug = vaug_p.tile([128, T, 65], F32R, tag="vaug")
                    nc.vector.tensor_tensor(
                        out=vaug[:, :, 0:DH], in0=v_sb[:, :, DH * h:DH * (h + 1)],
                        in1=g[:].rearrange("p (t one) -> p t one", one=1)
                             .broadcast_to([128, T, DH]),
                        op=ALU.mult)
                    nc.vector.tensor_copy(vaug[:, :, DH], g[:])

                    pctx = ps_ctx.tile([65, M], F32, tag="pctx")
                    for t in range(T):
                        nc.tensor.matmul(
                            pctx[:], vaug[:, t, :],
                            ek_sb[:, t, :],
                            start=(t == 0), stop=(t == T - 1), skip_group_check=True)
                    ctxs = small_p.tile([65, M], F32, tag="ctxs")
                    nc.vector.tensor_copy(ctxs[:], pctx[:])

                    ctxT = small_p.tile([128, 2, 65], F32R, tag="ctxT")
                    for mi in range(2):
                        pt = ps_o.tile([128, 65], F32, tag="po")
                        nc.tensor.transpose(pt[:], ctxs[:, 128 * mi:128 * (mi + 1)],
                                            ident_sb[0:65, 0:65])
                        nc.vector.tensor_copy(ctxT[:, mi, :], pt[:])

                    for j in range(J):
                        qpt = qpt_p.tile([128, 2, 512], F32R, tag="qpt")
                        for mi in range(2):
                            pq = ps_q.tile([128, 512], F32, tag="pq")
                            nc.tensor.matmul(
                                pq[:],
                                projT2_sb[DH:128, 128 * mi:128 * (mi + 1)],
                                qkT_sb[DH:128, 512 * j:512 * (j + 1)],
                                start=True, stop=True, skip_group_check=True)
                            nc.scalar.activation(qpt[:, mi, :], pq[:], AF.Exp, scale=DS)
                        po = ps_o.tile([65, 512], F32, tag="po")
                        for mi in range(2):
                            nc.tensor.matmul(
                                po[:], ctxT[:, mi, :],
                                qpt[:, mi, :],
                                start=(mi == 0), stop=(mi == 1), skip_group_check=True)
                        stag = stag_p.tile([65, 512], F32, tag="stag")
                        nc.vector.tensor_copy(stag[:], po[:])
                        nc.sync.dma_start(out=h_in[b, j, DH * h:DH * (h + 1), :],
                                          in_=stag[0:DH, :])
                        nc.sync.dma_start(out=h_in[b, j, 128 + h:129 + h, :],
                                          in_=stag[DH:DH + 1, :])

                nc.gpsimd.collective_compute(
                    "AllToAll", ALU.bypass, replica_groups=groups,
                    ins=[h_in[b]], outs=[h_out[b]])

                DF = 2 * NCORES * NS // 128
                nc.sync.dma_start(
                    out=den_scr[b].rearrange("(s h n) -> s h n", s=NCORES, h=2),
                    in_=h_out[b, :, 128:130, :])
                den128 = small_p.tile([128, DF], F32, tag="den128")
                nc.sync.dma_start(
                    out=den128[:], in_=den_scr[b].rearrange("(p f) -> p f", f=DF))
                dinv128 = small_p.tile([128, DF], F32, tag="dinv128")
                nc.vector.reciprocal(dinv128[:], den128[:])
                nc.sync.dma_start(
                    out=dinv_scr[b].rearrange("(p f) -> p f", f=DF), in_=dinv128[:])

                hgn = lin_p.tile([128, NCORES, NS], F32R, tag="hgn")
                for cc in range(NCORES):
                    hraw = stag_p.tile([128, NS], F32, tag="hraw")
                    nc.sync.dma_start(out=hraw[:], in_=h_out[b, cc, 0:128, :])
                    dinvB = stag_p.tile([128, NS], F32, tag="dinvB")
                    nc.sync.dma_start(
                        out=dinvB[:],
                        in_=dinv_scr[b, cc * 2 * NS:(cc + 1) * 2 * NS]
                            .rearrange("(h n) -> h n", h=2)
                            .unsqueeze(1)
                            .broadcast_to([2, DH, NS]))
                    nc.vector.tensor_tensor(out=hgn[:, cc, :], in0=hraw[:],
                                            in1=dinvB[:], op=ALU.mult)

                for nci in range(NS // 128):
                    for oh in range(2):
                        pl = ps_lin.tile([128, 512], F32, tag="pl")
                        for cc in range(NCORES):
                            nc.tensor.matmul(
                                pl[:],
                                hgn[:, cc, 128 * nci:128 * (nci + 1)],
                                WT_sb[:, cc, 512 * oh:512 * (oh + 1)],
                                start=(cc == 0), stop=(cc == NCORES - 1),
                                skip_group_check=True)
                        oc = outc_p.tile([128, 512], F32, tag="oc")
                        nc.scalar.activation(oc[:], pl[:], AF.Copy)
                        nc.sync.dma_start(
                            out=out_ext[b, 128 * nci:128 * (nci + 1),
                                        512 * oh:512 * (oh + 1)],
                            in_=oc[:])
    nc.compile()
    return nc


def _get_nc():
    if "nc" not in _CACHE:
        _CACHE["nc"] = _build()
    return _CACHE["nc"]


def _host_prep(q, k, v, W, proj):
    projT = np.ascontiguousarray(proj.T)
    projT2 = np.concatenate([projT, projT], axis=0)
    WTfull = np.ascontiguousarray(W.T).astype(np.float32)
    identity = np.eye(128, dtype=np.float32)
    in_maps = []
    for c in range(NCORES):
        lo = c * 128
        qc = q[:, :, lo:lo + 128]
        kc = k[:, :, lo:lo + 128]
        vc = v[:, :, lo:lo + 128]
        in_maps.append({
            "qT": np.ascontiguousarray(qc.reshape(B, N, 2, DH).transpose(0, 2, 3, 1)),
            "kT": np.ascontiguousarray(kc.reshape(B, N, 2, DH).transpose(0, 2, 3, 1)),
            "kn": np.ascontiguousarray(kc.reshape(B, T, 128, 128)),
            "vn": np.ascontiguousarray(vc.reshape(B, T, 128, 128)),
            "projT2": projT2,
            "WT": WTfull,
            "ident": identity,
        })
    return in_maps


def kernel(q, k, v, W, b, proj, _profile=False):
    q = np.asarray(q, np.float32)
    k = np.asarray(k, np.float32)
    v = np.asarray(v, np.float32)
    W = np.asarray(W, np.float32)
    b = np.asarray(b, np.float32)
    proj = np.asarray(proj, np.float32)

    nc = _get_nc()
    in_maps = _host_prep(q, k, v, W, proj)
    res = run_bass_kernel_spmd(nc, in_maps, list(range(NCORES)), trace=_profile)
    out = np.empty((B, N, D), dtype=np.float32)
    for c in range(NCORES):
        out[:, c * NS:(c + 1) * NS, :] = res.results[c]["out"]
    out += b
    if _profile:
        _CACHE["last_exec_time_ns"] = res.exec_time_ns
        _CACHE["last_profile_json"] = res.profile_json
    return out


# revision 4
# speedup vs baseline: 1.1337x; 1.1337x over previous
"""Performer (FAVOR+) multi-head fast-attention TRN2 kernel — self-contained.

Problem: B=4, N=4096, D=1024, H=16, M=256, DH=64.
Sharding: 2 heads per core (head-parallel attention) on 8 NeuronCores;
on-device AllToAll re-shards to sequence-parallel for the output Linear
(row-parallel, no partial sums); host stitches the 8 n-shards.

All Performer stabilizers that cancel in the num/den ratio are dropped
on device; the k-side row max and ||k||^2 factors are folded into v, so
the result matches the reference exactly up to float rounding.
"""
import contextlib
import sys

sys.path.insert(0, "/opt/trn_rl_repo")

import numpy as np

import concourse.bacc as bacc
import concourse.mybir as mybir
from concourse.tile import TileContext
from concourse.bass_utils import run_bass_kernel_spmd

F32 = mybir.dt.float32
F32R = mybir.dt.float32r
AF = mybir.ActivationFunctionType
ALU = mybir.AluOpType

NCORES = 8
B, N, D = 4, 4096, 1024
H, M, DH = 16, 256, 64
T = N // 128
J = N // 512
NS = N // NCORES
DS = float(DH) ** -0.25

_CACHE = {}


def _build():
    nc = bacc.Bacc(num_devices=NCORES)
    groups = [list(range(NCORES))]

    qT = nc.declare_dram_parameter("qT", [B, 2, DH, N], F32, isOutput=False)
    kT = nc.declare_dram_parameter("kT", [B, 2, DH, N], F32, isOutput=False)
    kn = nc.declare_dram_parameter("kn", [B, T, 128, 128], F32, isOutput=False)
    vn = nc.declare_dram_parameter("vn", [B, T, 128, 128], F32, isOutput=False)
    projT2 = nc.declare_dram_parameter("projT2", [128, M], F32, isOutput=False)
    WT = nc.declare_dram_parameter("WT", [D, D], F32, isOutput=False)
    ident = nc.declare_dram_parameter("ident", [128, 128], F32, isOutput=False)
    out_ext = nc.declare_dram_parameter("out", [B, NS, D], F32, isOutput=True)

    h_in = nc.dram_tensor("h_in", [B, NCORES, 130, NS], F32)
    h_out = nc.dram_tensor("h_out", [B, NCORES, 130, NS], F32)
    dinv_scr = nc.dram_tensor("dinv_scr", [B, 2 * NCORES * NS], F32)
    den_scr = nc.dram_tensor("den_scr", [B, 2 * NCORES * NS], F32)

    with TileContext(nc) as tc:
        with contextlib.ExitStack() as stk:
            const_p = stk.enter_context(tc.tile_pool(name="const", bufs=1))
            qkT_p = stk.enter_context(tc.tile_pool(name="qkT", bufs=2))
            knv_p = stk.enter_context(tc.tile_pool(name="knv", bufs=1))
            ek_p = stk.enter_context(tc.tile_pool(name="ek", bufs=1))
            small_p = stk.enter_context(tc.tile_pool(name="small", bufs=3))
            vaug_p = stk.enter_context(tc.tile_pool(name="vaug", bufs=1))
            qpt_p = stk.enter_context(tc.tile_pool(name="qpt", bufs=3))
            stag_p = stk.enter_context(tc.tile_pool(name="stag", bufs=3))
            lin_p = stk.enter_context(tc.tile_pool(name="lin", bufs=1))
            outc_p = stk.enter_context(tc.tile_pool(name="outc", bufs=3))
            ps_k = stk.enter_context(tc.tile_pool(name="psk", bufs=1, space="PSUM"))
            ps_q = stk.enter_context(tc.tile_pool(name="psq", bufs=1, space="PSUM"))
            ps_ctx = stk.enter_context(tc.tile_pool(name="psctx", bufs=1, space="PSUM"))
            ps_o = stk.enter_context(tc.tile_pool(name="pso", bufs=2, space="PSUM"))
            ps_lin = stk.enter_context(tc.tile_pool(name="pslin", bufs=1, space="PSUM"))

            projT2_sb = const_p.tile([128, M], F32R, tag="projT2")
            nc.sync.dma_start(out=projT2_sb[:], in_=projT2[:].bitcast(F32R))
            ident_sb = const_p.tile([128, 128], F32, tag="ident")
            nc.sync.dma_start(out=ident_sb[:], in_=ident[:])
            WT_sb = const_p.tile([128, NCORES, D], F32R, tag="WT")
            nc.sync.dma_start(out=WT_sb[:],
                              in_=WT[:].rearrange("(cc p) o -> p cc o", p=128).bitcast(F32R))

            for b in range(B):
                kn_sb = knv_p.tile([128, T, 128], F32, tag="kn")
                nc.sync.dma_start(out=kn_sb[:], in_=kn[b].rearrange("t p d -> p t d"))
                v_sb = knv_p.tile([128, T, 128], F32, tag="v")
                nc.sync.dma_start(out=v_sb[:], in_=vn[b].rearrange("t p d -> p t d"))

                kflat = kn_sb[:].rearrange("p t d -> p (t d)")
                nc.vector.scalar_tensor_tensor(
                    out=kflat, in0=kflat, scalar=0.0, in1=kflat,
                    op0=ALU.add, op1=ALU.mult)
                dn_raw = small_p.tile([128, T, 2], F32, tag="dn")
                nc.vector.tensor_reduce(
                    out=dn_raw[:],
                    in_=kn_sb[:].rearrange("p t (h d) -> p t h d", h=2),
                    axis=mybir.AxisListType.X, op=ALU.add)

                for h in range(2):
                    qkT_sb = qkT_p.tile([128, N], F32R, tag="qkT")
                    nc.sync.dma_start(out=qkT_sb[0:DH, :], in_=kT[b, h].bitcast(F32R))
                    nc.sync.dma_start(out=qkT_sb[DH:128, :], in_=qT[b, h].bitcast(F32R))

                    ek_sb = ek_p.tile([128, T, M], F32R, tag="ek")
                    me = small_p.tile([128, T], F32, tag="me")
                    for tb in range(T // 4):
                        pk4 = ps_k.tile([128, 4, M], F32, tag="pk")
                        for qq in range(4):
                            t = 4 * tb + qq
                            nc.tensor.matmul(
                                pk4[:, qq, :], qkT_sb[0:DH, 128 * t:128 * (t + 1)],
                                projT2_sb[0:DH, :],
                                start=True, stop=True, skip_group_check=True)
                        nc.scalar.activation(ek_sb[:, 4 * tb:4 * (tb + 1), :], pk4[:],
                                             AF.Exp, scale=DS)
                        nc.vector.tensor_reduce(
                            out=me[:, 4 * tb:4 * (tb + 1)],
                            in_=ek_sb[:, 4 * tb:4 * (tb + 1), :],
                            axis=mybir.AxisListType.X, op=ALU.max)
                    eg = small_p.tile([128, T], F32, tag="eg")
                    nc.scalar.activation(eg[:], dn_raw[:, :, h], AF.Exp,
                                         scale=-0.5 * DS * DS)
                    rme = small_p.tile([128, T], F32, tag="rme")
                    nc.vector.reciprocal(rme[:], me[:])
                    g = small_p.tile([128, T], F32, tag="g")
                    nc.vector.tensor_tensor(out=g[:], in0=eg[:], in1=rme[:],
                                            op=ALU.mult)

                    vaug = vaug_p.tile([128, T, 65], F32R, tag="vaug")
                    nc.vector.tensor_tensor(
                        out=vaug[:, :, 0:DH], in0=v_sb[:, :, DH * h:DH * (h + 1)],
                        in1=g[:].rearrange("p (t one) -> p t one", one=1)
                             .broadcast_to([128, T, DH]),
                        op=ALU.mult)
                    nc.vector.tensor_copy(vaug[:, :, DH], g[:])

                    pctx = ps_ctx.tile([65, M], F32, tag="pctx")
                    for t in range(T):
                        nc.tensor.matmul(
                            pctx[:], vaug[:, t, :],
                            ek_sb[:, t, :],
                            start=(t == 0), stop=(t == T - 1), skip_group_check=True)
                    ctxs = small_p.tile([65, M], F32, tag="ctxs")
                    nc.vector.tensor_copy(ctxs[:], pctx[:])

                    ctxT = small_p.tile([128, 2, 65], F32R, tag="ctxT")
                    for mi in range(2):
                        pt = ps_o.tile([128, 65], F32, tag="po")
                        nc.tensor.transpose(pt[:], ctxs[:, 128 * mi:128 * (mi + 1)],
                                            ident_sb[0:65, 0:65])
                        nc.vector.tensor_copy(ctxT[:, mi, :], pt[:])

                    for j in range(J):
                        qpt = qpt_p.tile([128, 2, 512], F32R, tag="qpt")
                        pq = ps_q.tile([128, 2, 512], F32, tag="pq")
                        for mi in range(2):
                            nc.tensor.matmul(
                                pq[:, mi, :],
                                projT2_sb[DH:128, 128 * mi:128 * (mi + 1)],
                                qkT_sb[DH:128, 512 * j:512 * (j + 1)],
                                start=True, stop=True, skip_group_check=True)
                        nc.scalar.activation(qpt[:], pq[:], AF.Exp, scale=DS)
                        po = ps_o.tile([65, 512], F32, tag="po")
                        for mi in range(2):
                            nc.tensor.matmul(
                                po[:], ctxT[:, mi, :],
                                qpt[:, mi, :],
                                start=(mi == 0), stop=(mi == 1), skip_group_check=True)
                        stag = stag_p.tile([65, 512], F32, tag="stag")
                        nc.vector.tensor_copy(stag[:], po[:])
                        nc.sync.dma_start(out=h_in[b, j, DH * h:DH * (h + 1), :],
                                          in_=stag[0:DH, :])
                        nc.sync.dma_start(out=h_in[b, j, 128 + h:129 + h, :],
                                          in_=stag[DH:DH + 1, :])

                nc.gpsimd.collective_compute(
                    "AllToAll", ALU.bypass, replica_groups=groups,
                    ins=[h_in[b]], outs=[h_out[b]])

                DF = 2 * NCORES * NS // 128
                nc.sync.dma_start(
                    out=den_scr[b].rearrange("(s h n) -> s h n", s=NCORES, h=2),
                    in_=h_out[b, :, 128:130, :])
                den128 = small_p.tile([128, DF], F32, tag="den128")
                nc.sync.dma_start(
                    out=den128[:], in_=den_scr[b].rearrange("(p f) -> p f", f=DF))
                dinv128 = small_p.tile([128, DF], F32, tag="dinv128")
                nc.vector.reciprocal(dinv128[:], den128[:])
                nc.sync.dma_start(
                    out=dinv_scr[b].rearrange("(p f) -> p f", f=DF), in_=dinv128[:])

                hgn = lin_p.tile([128, NCORES, NS], F32R, tag="hgn")
                for cc in range(NCORES):
                    hraw = stag_p.tile([128, NS], F32, tag="hraw")
                    nc.sync.dma_start(out=hraw[:], in_=h_out[b, cc, 0:128, :])
                    dinvB = stag_p.tile([128, NS], F32, tag="dinvB")
                    nc.sync.dma_start(
                        out=dinvB[:],
                        in_=dinv_scr[b, cc * 2 * NS:(cc + 1) * 2 * NS]
                            .rearrange("(h n) -> h n", h=2)
                            .unsqueeze(1)
                            .broadcast_to([2, DH, NS]))
                    nc.vector.tensor_tensor(out=hgn[:, cc, :], in0=hraw[:],
                                            in1=dinvB[:], op=ALU.mult)

                for nci in range(NS // 128):
                    for oh in range(2):
                        pl = ps_lin.tile([128, 512], F32, tag="pl")
                        for cc in range(NCORES):
                            nc.tensor.matmul(
                                pl[:],
                                hgn[:, cc, 128 * nci:128 * (nci + 1)],
                                WT_sb[:, cc, 512 * oh:512 * (oh + 1)],
                                start=(cc == 0), stop=(cc == NCORES - 1),
                                skip_group_check=True)
                        oc = outc_p.tile([128, 512], F32, tag="oc")
                        nc.scalar.activation(oc[:], pl[:], AF.Copy)
                        nc.sync.dma_start(
                            out=out_ext[b, 128 * nci:128 * (nci + 1),
                                        512 * oh:512 * (oh + 1)],
                            in_=oc[:])
    nc.compile()
    return nc


def _get_nc():
    if "nc" not in _CACHE:
        _CACHE["nc"] = _build()
    return _CACHE["nc"]


def _host_prep(q, k, v, W, proj):
    projT = np.ascontiguousarray(proj.T)
    projT2 = np.concatenate([projT, projT], axis=0)
    WTfull = np.ascontiguousarray(W.T).astype(np.float32)
    identity = np.eye(128, dtype=np.float32)
    in_maps = []
    for c in range(NCORES):
        lo = c * 128
        qc = q[:, :, lo:lo + 128]
        kc = k[:, :, lo:lo + 128]
        vc = v[:, :, lo:lo + 128]
        in_maps.append({
            "qT": np.ascontiguousarray(qc.reshape(B, N, 2, DH).transpose(0, 2, 3, 1)),
            "kT": np.ascontiguousarray(kc.reshape(B, N, 2, DH).transpose(0, 2, 3, 1)),
            "kn": np.ascontiguousarray(kc.reshape(B, T, 128, 128)),
            "vn": np.ascontiguousarray(vc.reshape(B, T, 128, 128)),
            "projT2": projT2,
            "WT": WTfull,
            "ident": identity,
        })
    return in_maps


def kernel(q, k, v, W, b, proj, _profile=False):
    q = np.asarray(q, np.float32)
    k = np.asarray(k, np.float32)
    v = np.asarray(v, np.float32)
    W = np.asarray(W, np.float32)
    b = np.asarray(b, np.float32)
    proj = np.asarray(proj, np.float32)

    nc = _get_nc()
    in_maps = _host_prep(q, k, v, W, proj)
    res = run_bass_kernel_spmd(nc, in_maps, list(range(NCORES)), trace=_profile)
    out = np.empty((B, N, D), dtype=np.float32)
    for c in range(NCORES):
        out[:, c * NS:(c + 1) * NS, :] = res.results[c]["out"]
    out += b
    if _profile:
        _CACHE["last_exec_time_ns"] = res.exec_time_ns
        _CACHE["last_profile_json"] = res.profile_json
    return out
